# revision 72
# baseline (speedup 1.0000x reference)
"""GCN layer (GCNConv + relu + dense + relu) on 8 Trainium2 NeuronCores.

Strategy
--------
Math: out = relu(relu(GCNConv(x)) @ W_dense + b_dense) with
GCNConv(x)[v] = dinv[v] * sum_{e: u->v} dinv[u] * (x W_gcn)[u] + b_gcn
(self-loops included as ordinary edges; dinv = rsqrt(indegree incl. self).)

Device plan (2 SPMD launches over 8 cores, nodes dst-sharded 12500/core):
  Launch A: per core, g = dinv_row * (x @ W_gcn), padded to [12800, 64] f32
            rows (tile-major). Host concatenates shards -> table [100000, 64].
  Launch B: per core, edges (incl. self-loops) sorted by (src-chunk,
            dst-tile) and padded to 128-multiples per (chunk, tile) group.
            dma_gather streams messages in that order; per 128-edge batch a
            DVE is_equal against an iota matrix builds the selection matrix
            S[e, slot] = (dst_slot[e] == slot) and a TensorE matmul
            S^T @ msgs accumulates straight into the destination tile's
            PSUM region ([128, 32] at bank t//16, offset (t%16)*32 -- all
            100 tiles = 3200 f32/partition live in PSUM at once, no HBM
            accumulator, no dma_scatter_add).  Epilogue per quad of tiles:
            dinv scale, +b_gcn, relu, @W_dense, +b_dense, relu via
            PE-transpose packed [128,128] tiles.

The batch/gather schedule is computed from the actual input on the host
(compile happens inside kernel() after seeing the edges) and shared by all
8 cores (per-(chunk,tile) capacity = max over cores); per-core slack is
filled with junk edges whose dst slot (999) matches no iota column, so
their S rows are all-zero and the matmul adds nothing.
"""

import sys

if "/opt/trn_rl_repo" not in sys.path:
    sys.path.insert(0, "/opt/trn_rl_repo")

from dataclasses import dataclass

import ml_dtypes
import numpy as np

import concourse.bacc as bacc
import concourse.mybir as mybir
from concourse import tile
from concourse.bass_utils import run_bass_kernel_spmd


@dataclass
class Cfg:
    n_cores: int = 8
    tiles: int = 100          # 128-row dst tiles per core (98 real + 2 pad)
    in_dim: int = 128
    net_dim: int = 32
    padf: int = 128           # table row width in bf16 (256B = gather minimum)
    n_chunks: int = 4         # source chunks (gather idx must fit int16)
    gather_batches: int = 8   # 128-edge batches per gather (1024 idx)
    nloc: int = 12500
    nqueues: int = 1          # SWDGE queues (HW exposes exactly one)

    @property
    def npad(self):
        return self.tiles * 128

    @property
    def n(self):
        return self.nloc * self.n_cores

    @property
    def ntab(self):
        # table rows incl. per-core pad rows; row = c*12800 + p*tiles + t
        # holds node c*nloc + t*128 + p (p-major so launch A writes one
        # contiguous DMA per core)
        return self.npad * self.n_cores

    @property
    def chunk(self):
        return self.ntab // self.n_chunks  # 25600


FULL = Cfg()
assert FULL.n == 100000 and FULL.chunk == 25600

JUNK_SLOT = 999.0  # matches no iota column 0..127 -> zero S row


def _f32(x):
    return np.ascontiguousarray(x, dtype=np.float32)


def wrap16(a):
    """Index array [n] -> [128, n//16] int16 layout dma_gather expects."""
    assert a.size % 16 == 0
    w = a.reshape(-1, 16).T
    return np.ascontiguousarray(np.tile(w, (8, 1)), dtype=np.int16)


# ---------------------------------------------------------------- launch A


def build_launch_a(cfg: Cfg):
    """g[p*T + t, 0:32] = dinv[t*128+p] * (x[t*128+p] @ W); one DMA each way.

    Host supplies x TRANSPOSED ([128 feat, 12800 rows], free on host), so
    each tile's matmul consumes an xT slice directly as lhsT -- no PE
    transposes, no PSUM round-trip.  The output table is written p-major
    (row p*T+t holds node t*128+p); launch B's gather indices absorb it.
    """
    nc = bacc.Bacc(
        "TRN2", target_bir_lowering=False, debug=False, num_devices=cfg.n_cores
    )
    T, K, F, PF = cfg.tiles, cfg.in_dim, cfg.net_dim, cfg.padf
    x_d = nc.dram_tensor("xT", [K, cfg.npad], mybir.dt.bfloat16, kind="ExternalInput")
    w_d = nc.dram_tensor("w", [K, F], mybir.dt.bfloat16, kind="ExternalInput")
    dinv_d = nc.dram_tensor("dinv", [128, T], mybir.dt.float32, kind="ExternalInput")
    g_d = nc.dram_tensor("g", [cfg.npad, PF], mybir.dt.bfloat16, kind="ExternalOutput")

    with tile.TileContext(nc) as tc:
        with (
            tc.tile_pool(name="const", bufs=1) as cpool,
            tc.tile_pool(name="xin", bufs=1) as xpool,
            tc.tile_pool(name="ph", bufs=4, space="PSUM") as php,
        ):
            w_t = cpool.tile([K, F], mybir.dt.bfloat16)
            dinv_t = cpool.tile([128, T], mybir.dt.float32)
            nc.sync.dma_start(out=w_t[:], in_=w_d[:])
            nc.sync.dma_start(out=dinv_t[:], in_=dinv_d[:])

            # quarter the in/out DMAs so the matmul loop overlaps both
            TQ = T // 4
            xT_q = [
                xpool.tile([K, TQ, 128], mybir.dt.bfloat16, name=f"x{i}")
                for i in range(4)
            ]
            g_q = [
                xpool.tile([128, TQ, PF], mybir.dt.bfloat16, name=f"g{i}")
                for i in range(4)
            ]
            for i in range(4):
                nc.sync.dma_start(
                    out=xT_q[i][:].rearrange("k t p -> k (t p)"),
                    in_=x_d[:, i * TQ * 128 : (i + 1) * TQ * 128],
                )
            for i in range(4):
                nc.vector.memset(g_q[i][:, :, F:], 0.0)
                for tl in range(TQ):
                    h_p = php.tile([128, F], mybir.dt.float32, tag="h")
                    nc.tensor.matmul(
                        h_p[:], xT_q[i][:, tl, :], w_t[:], start=True, stop=True
                    )
                    nc.scalar.activation(
                        g_q[i][:, tl, 0:F],
                        h_p[:],
                        mybir.ActivationFunctionType.Copy,
                        scale=dinv_t[:, i * TQ + tl : i * TQ + tl + 1],
                    )
                nc.sync.dma_start(
                    out=g_d.ap()
                    .flatten()
                    .rearrange("(p u) -> p u", p=128)[
                        :, i * TQ * PF : (i + 1) * TQ * PF
                    ],
                    in_=g_q[i][:].rearrange("p t f -> p (t f)"),
                )
    nc.compile()
    return nc


# ---------------------------------------------------------------- schedule


class Sched:
    """Shared (SPMD) batch/gather/matmul schedule from per-core edge counts.

    The per-(chunk, tile) slot capacity is the EXACT max edge count over
    cores (no 128-rounding); a 128-edge batch may therefore straddle group
    boundaries and contain edges of several tiles.  Each (batch, tile)
    intersection is a matmul "job" with its own slots column (slot value,
    or JUNK for edges of other tiles / per-core slack).

    gathers: (chunk, b0, nb) -- gather covers batches [b0, b0+nb).
    jobs_by_batch[b]: list of (tile, job_idx).
    bank_stop[job_idx]: stop=True on the last matmul into each psum bank.
    """

    def __init__(self, cfg: Cfg, counts):
        # counts: [n_cores, n_chunks, tiles] real-edge counts.
        # Each core packs its edges back-to-back at its OWN cumsum positions
        # (no per-group shared capacity); only the per-chunk batch budget is
        # max-aligned across cores. A batch's job set is the UNION over
        # cores of tiles overlapping it; per-core slack reads as JUNK slots.
        chunk_tot = counts.sum(axis=2)  # [cores, chunks]
        nb_ch = np.ceil(chunk_tot.max(axis=0) / 128.0).astype(np.int64)
        self.chunk_batch0 = np.concatenate([[0], np.cumsum(nb_ch)])
        self.nbatch = int(self.chunk_batch0[-1])
        self.total_slots = self.nbatch * 128
        # per-core slot boundaries of each (ch, t) group
        starts = np.zeros(
            (cfg.n_cores, cfg.n_chunks, cfg.tiles + 1), dtype=np.int64
        )
        for ch in range(cfg.n_chunks):
            base = self.chunk_batch0[ch] * 128
            starts[:, ch, 0] = base
            starts[:, ch, 1:] = base + np.cumsum(counts[:, ch, :], axis=1)
        self.starts = starts

        tiles_of_batch = [set() for _ in range(self.nbatch)]
        for c in range(cfg.n_cores):
            for ch in range(cfg.n_chunks):
                for t in range(cfg.tiles):
                    s0, s1 = starts[c, ch, t], starts[c, ch, t + 1]
                    if s1 == s0:
                        continue
                    for b in range(s0 // 128, -(-s1 // 128)):
                        tiles_of_batch[b].add(t)
        # jobs numbered batch-major so each gather's jobs form a contiguous
        # slots range (one batched DVE is_equal builds all its S matrices)
        self.jobs_by_batch = []
        self.job_lut = np.full((self.nbatch, cfg.tiles), -1, dtype=np.int64)
        njobs = 0
        for b in range(self.nbatch):
            entry = []
            for t in sorted(tiles_of_batch[b]):
                self.job_lut[b, t] = njobs
                entry.append((t, njobs))
                njobs += 1
            self.jobs_by_batch.append(entry)
        self.njobs = njobs

        self.gathers = []
        self.chunk_gather0 = []
        for ch in range(cfg.n_chunks):
            self.chunk_gather0.append(len(self.gathers))
            b0, b1 = self.chunk_batch0[ch], self.chunk_batch0[ch + 1]
            k = b0
            while k < b1:
                sz = min(cfg.gather_batches, b1 - k)
                self.gathers.append((ch, k, sz))
                k += sz
        self.ngather = len(self.gathers)
        self.chunk_gather0.append(self.ngather)
        # per-gather contiguous job range [j0, j0+nj)
        self.gather_jobs = []
        for ch, b0, nbg in self.gathers:
            js = [jx for b in range(b0, b0 + nbg) for _, jx in self.jobs_by_batch[b]]
            assert js == list(range(js[0], js[0] + len(js)))
            self.gather_jobs.append((js[0], len(js)))
        self.njmax = max(nj for _, nj in self.gather_jobs)
        # stop=True on the last matmul into each PSUM bank
        last_per_bank = {}
        for b in range(self.nbatch):
            for t, jx in self.jobs_by_batch[b]:
                last_per_bank[t // 16] = jx
        stops = set(last_per_bank.values())
        self.bank_stop = [jx in stops for jx in range(njobs)]
        # gather after which each PSUM bank (and its quads) is complete,
        # so per-quad epilogues can be emitted inline and overlap the
        # remaining gather stream
        gather_of_batch = np.zeros(self.nbatch, dtype=np.int64)
        for gi, (ch, b0, nbg) in enumerate(self.gathers):
            gather_of_batch[b0 : b0 + nbg] = gi
        bank_last = np.zeros(cfg.tiles // 16 + 1, dtype=np.int64)
        for b in range(self.nbatch):
            for t, _ in self.jobs_by_batch[b]:
                bank_last[t // 16] = gather_of_batch[b]
        self.quads_after_gather = [[] for _ in range(self.ngather)]
        for q in range(cfg.tiles // 4):
            self.quads_after_gather[int(bank_last[q // 4])].append(q)

        # stop=True on the last matmul into each PSUM bank (bank = tile//16)
        last_per_bank = {}
        for b in range(self.nbatch):
            for t, jx in self.jobs_by_batch[b]:
                last_per_bank[t // 16] = jx
        stops = set(last_per_bank.values())
        self.bank_stop = [jx in stops for jx in range(njobs)]


# ---------------------------------------------------------------- launch B


def build_launch_b(cfg: Cfg, sched: Sched):
    nc = bacc.Bacc(
        "TRN2", target_bir_lowering=False, debug=False, num_devices=cfg.n_cores
    )
    T, F, PF = cfg.tiles, cfg.net_dim, cfg.padf
    NJ, NG = sched.njobs, sched.ngather
    GW = cfg.gather_batches * 8  # int16 idx columns per gather slot

    g_d = nc.dram_tensor("g", [cfg.ntab, PF], mybir.dt.bfloat16, kind="ExternalInput")
    gown_d = nc.dram_tensor(
        "gown", [cfg.npad, PF], mybir.dt.bfloat16, kind="ExternalInput"
    )
    src_d = nc.dram_tensor("src_i", [128, NG * GW], mybir.dt.int16, kind="ExternalInput")
    slot_d = nc.dram_tensor("slots", [128, NJ], mybir.dt.bfloat16, kind="ExternalInput")
    slot2_d = nc.dram_tensor(
        "slots2", [128, NJ], mybir.dt.bfloat16, kind="ExternalInput"
    )
    dinv_d = nc.dram_tensor("dinv", [128, T], mybir.dt.float32, kind="ExternalInput")
    iota_d = nc.dram_tensor("iota", [128, 128], mybir.dt.bfloat16, kind="ExternalInput")
    bg_d = nc.dram_tensor("bg", [F, 1], mybir.dt.float32, kind="ExternalInput")
    wd_d = nc.dram_tensor("wd", [F, F], mybir.dt.float32, kind="ExternalInput")
    bd_d = nc.dram_tensor("bd", [F, 1], mybir.dt.float32, kind="ExternalInput")
    eye_d = nc.dram_tensor("eye", [128, 128], mybir.dt.float32, kind="ExternalInput")
    out_d = nc.dram_tensor(
        "out", [cfg.npad, F], mybir.dt.float32, kind="ExternalOutput"
    )

    with tile.TileContext(nc) as tc:
        with (
            tc.tile_pool(name="const", bufs=1) as cpool,
            tc.tile_pool(name="msg", bufs=8) as mpool,
            tc.tile_pool(name="sel", bufs=3) as spool,
            tc.tile_pool(name="epi", bufs=1) as epool,
            tc.tile_pool(name="acc", bufs=1, space="PSUM") as accp,
            tc.tile_pool(name="pt", bufs=1, space="PSUM") as ptp,
        ):
            # ---- constants / preloaded index data.  DMA order is the sync
            # queue order: the first gather only needs chunk-0 indices,
            # slots, and iota, so those load first; chunks 1-3 and the
            # epilogue constants stream in under the early gathers.
            iota_t = cpool.tile([128, 128], mybir.dt.bfloat16)
            eye_t = cpool.tile([128, 128], mybir.dt.float32)
            dinv_t = cpool.tile([128, T], mybir.dt.float32)
            slots_t = cpool.tile([128, NJ], mybir.dt.bfloat16)
            slots2_t = cpool.tile([128, NJ], mybir.dt.bfloat16)
            gown_t = cpool.tile([128, T, PF], mybir.dt.bfloat16)
            gownf_t = cpool.tile([128, T, F], mybir.dt.float32)
            srcs_ch = []
            for ch in range(cfg.n_chunks):
                g0, g1 = sched.chunk_gather0[ch], sched.chunk_gather0[ch + 1]
                srcs_ch.append(
                    cpool.tile([128, g1 - g0, GW], mybir.dt.int16, name=f"src{ch}")
                )

            def load_srcs(ch):
                g0, g1 = sched.chunk_gather0[ch], sched.chunk_gather0[ch + 1]
                nc.sync.dma_start(
                    out=srcs_ch[ch][:].rearrange("p g w -> p (g w)"),
                    in_=src_d[:, g0 * GW : g1 * GW],
                )

            load_srcs(0)
            nc.sync.dma_start(out=slots_t[:], in_=slot_d[:])
            nc.sync.dma_start(out=slots2_t[:], in_=slot2_d[:])
            nc.sync.dma_start(out=iota_t[:], in_=iota_d[:])
            for ch in range(1, cfg.n_chunks):
                load_srcs(ch)
            nc.sync.dma_start(out=eye_t[:], in_=eye_d[:])
            nc.sync.dma_start(out=dinv_t[:], in_=dinv_d[:])
            nc.sync.dma_start(
                out=gown_t[:].rearrange("p t f -> p (t f)"),
                in_=gown_d.ap().flatten().rearrange("(p u) -> p u", p=128),
            )
            nc.vector.tensor_copy(out=gownf_t[:], in_=gown_t[:, :, 0:F])
            wpack_t = cpool.tile([128, 128], mybir.dt.float32)
            nc.vector.memset(wpack_t[:], 0.0)
            bg_t = cpool.tile([128, 1], mybir.dt.float32)
            bd_t = cpool.tile([128, 1], mybir.dt.float32)
            for grp in range(4):
                sl = slice(F * grp, F * grp + F)
                nc.sync.dma_start(out=wpack_t[sl, sl], in_=wd_d[:])
                nc.sync.dma_start(out=bg_t[sl, :], in_=bg_d[:])
                nc.sync.dma_start(out=bd_t[sl, :], in_=bd_d[:])

            # ---- per-dst-tile psum accumulators: tile t lives at
            # bank t//16, f32 offset (t%16)*32.  start=True clears the WHOLE
            # bank's has_written bits, so each bank gets exactly one
            # start=True zero-init matmul; every edge matmul accumulates
            # with start=False (overwrite-where-clear handles nothing: the
            # init sets has_written for every element).
            zeros_t = cpool.tile([128, 512], mybir.dt.float32)
            nc.vector.memset(zeros_t[:], 0.0)
            banks = []
            for i in range((T + 15) // 16):
                bank = accp.tile([128, 512], mybir.dt.float32, name=f"bank{i}")
                nc.tensor.matmul(
                    bank[:], eye_t[:], zeros_t[:], start=True, stop=False
                )
                banks.append(bank)

            # ---- epilogue resources (quad epilogues are emitted inline,
            # right after the gather that completes their PSUM bank, so
            # they overlap the remaining gather stream)
            out_stage = epool.tile([128, T, F], mybir.dt.float32)
            epi_p = ptp.tile([128, 384], mybir.dt.float32)

            def emit_quad(q):
                bank = banks[(4 * q) // 16]
                boff = ((4 * q) % 16) * 32
                hq_t = epool.tile([128, 4, F], mybir.dt.float32, tag="hq")
                # self-loop contribution comes from the core's own table
                # rows instead of gather edges: (psum + g_own) * dinv
                nc.vector.tensor_tensor(
                    out=hq_t[:],
                    in0=bank[:, boff : boff + 128].rearrange(
                        "p (t f) -> p t f", t=4
                    ),
                    in1=gownf_t[:, 4 * q : 4 * q + 4, :],
                    op=mybir.AluOpType.add,
                )
                nc.vector.tensor_tensor(
                    out=hq_t[:],
                    in0=hq_t[:],
                    in1=dinv_t[:, 4 * q : 4 * q + 4]
                    .unsqueeze(2)
                    .broadcast_to((128, 4, F)),
                    op=mybir.AluOpType.mult,
                )
                hqT_p = epi_p[:, 0:128]
                nc.tensor.transpose(
                    hqT_p, hq_t[:].rearrange("p t f -> p (t f)"), eye_t[:]
                )
                h1T_t = spool.tile([128, 128], mybir.dt.float32, tag="h1T")
                nc.scalar.activation(
                    h1T_t[:], hqT_p, mybir.ActivationFunctionType.Relu,
                    bias=bg_t[:],
                )
                h2T_p = epi_p[:, 128:256]
                nc.tensor.matmul(
                    h2T_p, wpack_t[:], h1T_t[:], start=True, stop=True
                )
                h2T_t = spool.tile([128, 128], mybir.dt.float32, tag="h2T")
                nc.scalar.activation(
                    h2T_t[:], h2T_p, mybir.ActivationFunctionType.Relu,
                    bias=bd_t[:],
                )
                oT_p = epi_p[:, 256:384]
                nc.tensor.transpose(oT_p, h2T_t[:], eye_t[:])
                nc.vector.tensor_copy(
                    out=out_stage[:, 4 * q : 4 * q + 4, :].rearrange(
                        "p t f -> p (t f)"
                    ),
                    in_=oT_p,
                )
                # out row p*T + t = local node (t*128 + p); host untangles.
                nc.sync.dma_start(
                    out=out_d.ap()
                    .flatten()
                    .rearrange("(p u) -> p u", p=128)[:, 128 * q : 128 * (q + 1)],
                    in_=out_stage[:, 4 * q : 4 * q + 4, :].rearrange(
                        "p t f -> p (t f)"
                    ),
                )

            # ---- edge phase
            for gi, (ch, b0, nbg) in enumerate(sched.gathers):
                size = nbg * 128
                msg_t = mpool.tile(
                    [128, cfg.gather_batches, PF], mybir.dt.bfloat16, tag="m"
                )
                lo = ch * cfg.chunk
                hi = min(lo + cfg.chunk, cfg.ntab)
                nc.gpsimd.dma_gather(
                    msg_t[:, :nbg, :],
                    g_d[lo:hi, :],
                    srcs_ch[ch][:, gi - sched.chunk_gather0[ch], : size // 16],
                    size,
                    size,
                    PF,
                    queue_num=gi % cfg.nqueues,
                )
                # one batched is_equal builds every S matrix of this gather;
                # a second compare + add makes rows two-hot where a folded
                # duplicate (same src, same tile) rides along
                j0, nj = sched.gather_jobs[gi]
                s_all = spool.tile(
                    [128, sched.njmax, 128], mybir.dt.bfloat16, tag="S"
                )
                s2_all = spool.tile(
                    [128, sched.njmax, 128], mybir.dt.bfloat16, tag="S2"
                )
                nc.vector.tensor_tensor(
                    out=s_all[:, :nj, :],
                    in0=iota_t[:].unsqueeze(1).broadcast_to((128, nj, 128)),
                    in1=slots_t[:, j0 : j0 + nj]
                    .unsqueeze(2)
                    .broadcast_to((128, nj, 128)),
                    op=mybir.AluOpType.is_equal,
                )
                nc.vector.tensor_tensor(
                    out=s2_all[:, :nj, :],
                    in0=iota_t[:].unsqueeze(1).broadcast_to((128, nj, 128)),
                    in1=slots2_t[:, j0 : j0 + nj]
                    .unsqueeze(2)
                    .broadcast_to((128, nj, 128)),
                    op=mybir.AluOpType.is_equal,
                )
                nc.vector.tensor_tensor(
                    out=s_all[:, :nj, :],
                    in0=s_all[:, :nj, :],
                    in1=s2_all[:, :nj, :],
                    op=mybir.AluOpType.add,
                )
                for j in range(nbg):
                    b = b0 + j
                    for t, jx in sched.jobs_by_batch[b]:
                        off = (t % 16) * 32
                        nc.tensor.matmul(
                            banks[t // 16][:, off : off + F],
                            s_all[:, jx - j0, :],
                            msg_t[:, j, 0:F],
                            start=False,
                            stop=sched.bank_stop[jx],
                        )
                for q in sched.quads_after_gather[gi]:
                    emit_quad(q)
    nc.compile()
    return nc


# ---------------------------------------------------------------- host side


def host_prep(x, edge_index, W_gcn, b_gcn, W_dense, b_dense, cfg: Cfg):
    n, nloc = cfg.n, cfg.nloc
    row = np.asarray(edge_index[0]).astype(np.int64)
    col = np.asarray(edge_index[1]).astype(np.int64)
    deg = np.bincount(col, minlength=n).astype(np.float64) + 1.0  # + self-loop
    dinv_full = (1.0 / np.sqrt(deg)).astype(np.float32)

    eye = np.eye(128, dtype=np.float32)
    iota = np.broadcast_to(
        np.arange(128, dtype=np.float32)[None, :], (128, 128)
    ).copy()
    W_gcn = _f32(W_gcn)
    b_gcn = _f32(b_gcn).reshape(cfg.net_dim, 1)
    W_dense = _f32(W_dense)
    b_dense = _f32(b_dense).reshape(cfg.net_dim, 1)
    x = _f32(x)

    # self-loops are folded into the epilogue (gown), not gathered as edges
    owner = col // nloc

    # gather index of a source node in the p-major table written by launch A:
    # node s = c2*nloc + v lives at table row c2*npad + (v%128)*T + v//128
    sc = row // nloc
    sv = row % nloc
    tabrow = sc * cfg.npad + (sv % 128) * cfg.tiles + sv // 128

    per_core = []
    counts = np.zeros((cfg.n_cores, cfg.n_chunks, cfg.tiles), dtype=np.int64)
    for c in range(cfg.n_cores):
        m = owner == c
        srcs = tabrow[m]
        dstl = col[m] - c * nloc
        ch = srcs // cfg.chunk
        t = dstl // 128
        # sort by (chunk, tile, src) so equal-source runs are adjacent
        order = np.lexsort((srcs, t, ch))
        srcs, dstl, ch, t = srcs[order], dstl[order], ch[order], t[order]
        # fold duplicate (src, tile) pairs: edges sharing a source within
        # the same (chunk, tile) share one gathered row -- the second
        # edge's slot rides along as a two-hot S row (slot2); ~2% fewer
        # gather indices.  Runs of length L keep ceil(L/2) edges.
        n_e = srcs.size
        same = np.zeros(n_e, dtype=bool)
        same[1:] = (
            (srcs[1:] == srcs[:-1]) & (t[1:] == t[:-1]) & (ch[1:] == ch[:-1])
        )
        idx = np.arange(n_e)
        run_start = np.maximum.accumulate(np.where(~same, idx, 0))
        fold = ((idx - run_start) % 2) == 1
        slotv = (dstl % 128).astype(np.float32)
        slot2 = np.full(n_e, JUNK_SLOT, dtype=np.float32)
        has2 = np.zeros(n_e, dtype=bool)
        has2[:-1] = fold[1:]
        slot2[has2] = slotv[fold]
        keep = ~fold
        srcs, dstl, ch, t = srcs[keep], dstl[keep], ch[keep], t[keep]
        slotv, slot2 = slotv[keep], slot2[keep]
        np.add.at(counts[c], (ch, t), 1)
        per_core.append((srcs, slotv, slot2, ch, t))

    sched = Sched(cfg, counts)
    NJ, NG, GW = sched.njobs, sched.ngather, cfg.gather_batches * 8
    total_slots = sched.total_slots

    in_a, in_b = [], []
    for c in range(cfg.n_cores):
        srcs, slotv, slot2v, ch, t = per_core[c]
        # group-local position: edges are (chunk, tile)-sorted, so position
        # within group = running index minus group start; groups sit at
        # this core's own cumsum positions (sched.starts[c])
        gidx = ch * cfg.tiles + t
        rel = np.zeros(cfg.n_chunks * cfg.tiles, dtype=np.int64)
        cnt = np.bincount(gidx, minlength=cfg.n_chunks * cfg.tiles)
        rel[1:] = np.cumsum(cnt)[:-1]
        pos_in_grp = np.arange(srcs.size) - rel[gidx]
        slot_pos = sched.starts[c, :, :-1].reshape(-1)[gidx] + pos_in_grp

        all_src = np.zeros(total_slots, dtype=np.int64)
        all_src[slot_pos] = srcs - ch * cfg.chunk

        # slots column per (batch, tile) job: slot value for edges of that
        # job's tile inside that batch, JUNK elsewhere; slots2 carries the
        # folded duplicate's slot (two-hot S rows)
        batch_of = slot_pos // 128
        p_of = slot_pos % 128
        jx_of = sched.job_lut[batch_of, t]
        assert (jx_of >= 0).all()
        slots = np.full((128, NJ), JUNK_SLOT, dtype=np.float32)
        slots[p_of, jx_of] = slotv
        slots2 = np.full((128, NJ), JUNK_SLOT, dtype=np.float32)
        slots2[p_of, jx_of] = slot2v

        src_i = np.zeros((128, NG, GW), dtype=np.int16)
        for gi, (chg, b0, nbg) in enumerate(sched.gathers):
            seg = all_src[b0 * 128 : (b0 + nbg) * 128]
            src_i[:, gi, : nbg * 8] = wrap16(seg)

        # dinv in both layouts: tile-major for launch A scaling rows
        # [t*128+s], and the same layout works for launch B epilogue.
        dloc = np.ones(cfg.npad, dtype=np.float32)
        dloc[:nloc] = dinv_full[c * nloc : (c + 1) * nloc]
        dinv_tm = dloc.reshape(cfg.tiles, 128).T.copy()  # [128, T]

        xpad = np.zeros((cfg.npad, cfg.in_dim), dtype=np.float32)
        xpad[:nloc] = x[c * nloc : (c + 1) * nloc]
        in_a.append(
            {
                "xT": np.ascontiguousarray(xpad.T).astype(ml_dtypes.bfloat16),
                "w": W_gcn.astype(ml_dtypes.bfloat16),
                "dinv": dinv_tm,
            }
        )
        in_b.append(
            {
                "src_i": np.ascontiguousarray(
                    src_i.reshape(128, NG * GW)
                ),
                "slots": slots.astype(ml_dtypes.bfloat16),
                "slots2": slots2.astype(ml_dtypes.bfloat16),
                "dinv": dinv_tm,
                "iota": iota.astype(ml_dtypes.bfloat16),
                "bg": b_gcn,
                "wd": W_dense,
                "bd": b_dense,
                "eye": eye,
            }
        )
    return in_a, in_b, sched


def assemble_table(res_a, cfg: Cfg):
    # table rows are p-major per core (incl. pad rows); gather indices
    # already point at the right rows, so a plain concat is enough.
    return np.concatenate(
        [res_a[c]["g"] for c in range(cfg.n_cores)], axis=0
    )


def assemble_out(res_b, cfg: Cfg):
    outs = []
    for c in range(cfg.n_cores):
        a = res_b[c]["out"].reshape(128, cfg.tiles, cfg.net_dim)
        outs.append(a.transpose(1, 0, 2).reshape(cfg.npad, cfg.net_dim)[: cfg.nloc])
    return np.concatenate(outs, axis=0)


_NC_CACHE = {}


def _get_ncs(cfg: Cfg, sched: Sched):
    key = (cfg.n, cfg.tiles, sched.nbatch, sched.ngather, sched.njobs)
    if key not in _NC_CACHE:
        _NC_CACHE[key] = (build_launch_a(cfg), build_launch_b(cfg, sched))
    return _NC_CACHE[key]


def _add_table(in_b, table, cfg: Cfg):
    for c, m in enumerate(in_b):
        m["g"] = table
        m["gown"] = table[c * cfg.npad : (c + 1) * cfg.npad]


def kernel(x, edge_index, W_gcn, b_gcn, W_dense, b_dense):
    cfg = FULL
    in_a, in_b, sched = host_prep(
        x, edge_index, W_gcn, b_gcn, W_dense, b_dense, cfg
    )
    nc_a, nc_b = _get_ncs(cfg, sched)
    core_ids = list(range(cfg.n_cores))
    res_a = run_bass_kernel_spmd(nc_a, in_a, core_ids).results
    table = assemble_table(res_a, cfg)
    _add_table(in_b, table, cfg)
    res_b = run_bass_kernel_spmd(nc_b, in_b, core_ids).results
    return assemble_out(res_b, cfg)
